# revision 29
# baseline (speedup 1.0000x reference)
"""Multi-head attention (QKV projection + masked softmax + PV) on 8 TRN2
NeuronCores.

Sharding: data-parallel over batch (B=2 -> 2 groups of 4 cores), tensor
parallel over heads (16 heads -> 4 heads per core). Each core computes full
F x T attention for its 4 heads.

Per-core device algorithm (kept transposed so the softmax reduction lands on
the TensorE contraction dim; all matmuls bf16/fp16, PSUM fp32):
  Q^T[h,f] = wq^T @ from^T        K^T[h,t] = wk^T @ to^T
  V[t,hh]  = to^T^T @ wv   (+ a ones column per head for the softmax sums)
  S^T[t,f] = K^T(stationary, zero-padded to K=128) x Q^T(moving)
  E = exp(S^T/8) (ScalarE, PSUM->SBUF, bf16);  E *= mask^T (bf16)
  ctx^T[h,f] (+ sums[f] via the ones column) = sum_t V x E
  out = ctx^T * (1/sums)   reciprocals batched 4-per-j; broadcast down the
                           h-partitions via a selector-row ones matmul
Every matmul keeps tile_size (128,128) -- K=1 products are zero-padded to
K=128 via selector rows -- so the PE array never drains for a mode switch,
and PV is emitted one quad behind S so the PE stream never stalls long
enough for the HAM clock gate to re-throttle.

Host does the cheap layout work: pre-transposes from/to/mask (bf16), slices
weights per head group, transposes the [4,64,2048] per-core results back into
[B,F,N,H].
"""

import os
import sys

for _p in ("/opt/trn_rl_repo",):
    if os.path.isdir(_p) and _p not in sys.path:
        sys.path.insert(0, _p)

import numpy as np
import ml_dtypes

import concourse.tile as tile
from concourse import bacc, mybir
from concourse.bass_utils import run_bass_kernel_spmd

B, F, T, D, N, H = 2, 2048, 2048, 1024, 16, 64
NCORES = 8
HPC = N // (NCORES // B)  # heads per core = 4
NG = HPC // 2             # 128-partition head groups (2 heads each) = 2
FB = 512                  # f-block (psum bank width in fp32)
NJ = F // FB              # 4
NT = T // 128             # 16 t-tiles
NK = D // 128             # 8 contraction tiles
HP1 = H + 1               # head V columns incl. the ones column

F32 = mybir.dt.float32
F16 = mybir.dt.float16
BF16 = mybir.dt.bfloat16


def _phase1_kv(nc, tc, p1, ps1, tensors):
    """Load inputs, compute K^T (parity-split) and V, both per t-block as the
    toT chunks land. Q^T is emitted later, interleaved with the attention
    j-loop, so ScalarE starts exp'ing as soon as K^T and Q^T(j0) exist."""
    (fromT, toT, wq, wk, wv) = tensors["dram"]
    (QT, KTe, KTo, Vsb, bias_sb, bv_sb, vones_sb) = tensors["sbuf"]
    toT_sb = p1.tile([128, NK, T], BF16)
    fromT_sb = p1.tile([128, NK, F], BF16)
    wq_sb = p1.tile([128, NK, HPC * H], BF16)
    wk_sb = p1.tile([128, NK, HPC * H], BF16)
    wv_sb = p1.tile([128, NK, HPC * H], BF16)
    # critical path (HW-DGE queue): wk + toT t-block0 + wv + wq
    for k in range(NK):
        nc.sync.dma_start(wk_sb[:, k, :], wk[k * 128:(k + 1) * 128, :])
        nc.sync.dma_start(toT_sb[:, k, 0:FB], toT[k * 128:(k + 1) * 128, 0:FB])
    nc.sync.dma_start(wv_sb[:], wv[:].rearrange("(k p) m -> p k m", p=128))
    for k in range(NK):
        nc.sync.dma_start(wq_sb[:, k, :], wq[k * 128:(k + 1) * 128, :])
    # SW-DGE queue: fromT (for Q^T at the start of the attention loop),
    # then the remaining toT t-blocks
    for k in range(NK):
        nc.gpsimd.dma_start(fromT_sb[:, k, :], fromT[k * 128:(k + 1) * 128, :])
    for tb in range(1, NJ):
        for k in range(NK):
            nc.gpsimd.dma_start(
                toT_sb[:, k, tb * FB:(tb + 1) * FB],
                toT[k * 128:(k + 1) * 128, tb * FB:(tb + 1) * FB],
            )

    kv = dict(toT_sb=toT_sb, wk_sb=wk_sb, wv_sb=wv_sb)
    _emit_kv(nc, ps1, kv, tensors["sbuf"], 0)
    return wq_sb, fromT_sb, kv


def _emit_kv(nc, ps1, kv, sbuf, tb):
    (QT, KTe, KTo, Vsb, bias_sb, bv_sb, vones_sb) = sbuf
    toT_sb, wk_sb, wv_sb = kv["toT_sb"], kv["wk_sb"], kv["wv_sb"]
    for g in range(NG):
        ps_qk = ps1.tile([128, FB], F32, tag="qk")
        for k in range(NK):
            nc.tensor.matmul(
                ps_qk[:],
                wk_sb[:, k, g * 128:(g + 1) * 128],
                toT_sb[:, k, tb * FB:(tb + 1) * FB],
                start=(k == 0),
                stop=(k == NK - 1),
            )
        nc.vector.tensor_scalar_add(
            KTe[0:64, g, tb * FB:(tb + 1) * FB],
            ps_qk[0:64, :],
            bias_sb[0:64, NG + g:NG + g + 1],
        )
        nc.vector.tensor_scalar_add(
            KTo[64:128, g, tb * FB:(tb + 1) * FB],
            ps_qk[64:128, :],
            bias_sb[64:128, NG + g:NG + g + 1],
        )
    for ti in range(tb * 4, tb * 4 + 4):
        ps_v = ps1.tile([128, HPC * H], F32, tag="qk", name="ps_v")
        for k in range(NK):
            nc.tensor.matmul(
                ps_v[:],
                toT_sb[:, k, ti * 128:(ti + 1) * 128],
                wv_sb[:, k, :],
                start=(k == 0),
                stop=False,
            )
        nc.tensor.matmul(ps_v[:], vones_sb[:], bv_sb[:], start=False, stop=True)
        for nl in range(HPC):
            nc.vector.tensor_copy(
                Vsb[:, ti, nl * HP1:nl * HP1 + H],
                ps_v[:, nl * H:(nl + 1) * H],
            )


def _emit_qt(nc, ps1, wq_sb, fromT_sb, QT, bias_sb, j):
    for g in range(NG):
        ps_qk = ps1.tile([128, FB], F32, tag="qk")
        for k in range(NK):
            nc.tensor.matmul(
                ps_qk[:],
                wq_sb[:, k, g * 128:(g + 1) * 128],
                fromT_sb[:, k, j * FB:(j + 1) * FB],
                start=(k == 0),
                stop=(k == NK - 1),
            )
        nc.vector.tensor_scalar_add(
            QT[:, g, j * FB:(j + 1) * FB],
            ps_qk[:],
            bias_sb[:, g:g + 1],
        )


def _program():
    nc = bacc.Bacc(None, target_bir_lowering=False)
    fromT = nc.declare_dram_parameter("fromT", [D, F], BF16, isOutput=False)
    toT = nc.declare_dram_parameter("toT", [D, T], BF16, isOutput=False)
    maskT = nc.declare_dram_parameter("maskT", [T, F], BF16, isOutput=False)
    wq = nc.declare_dram_parameter("wq", [D, HPC * H], BF16, isOutput=False)
    wk = nc.declare_dram_parameter("wk", [D, HPC * H], BF16, isOutput=False)
    wv = nc.declare_dram_parameter("wv", [D, HPC * H], BF16, isOutput=False)
    bqk = nc.declare_dram_parameter("bqk", [128, 2 * NG], F32, isOutput=False)
    # bv padded to K=128 (row 0 = bv, rest zero) for a mode-switch-free matmul
    bv_pad = nc.declare_dram_parameter("bv_pad", [128, HPC * H], BF16, isOutput=False)
    # all-ones row 0 (rest zero): stationary operand of the bv matmul
    vones = nc.declare_dram_parameter("vones", [128, 128], BF16, isOutput=False)
    # selector blocks: ones_bc[k, nn, m] = (k == nn), broadcast matmul lhsT
    ones_bc = nc.declare_dram_parameter("ones_bc", [128, HPC, 128], F16, isOutput=False)
    out_ctx = nc.declare_dram_parameter("out_ctx", [HPC, H, F], F32, isOutput=True)

    with tile.TileContext(nc) as tc:
        with tc.tile_pool(name="persist", bufs=1) as persist:
            QT = persist.tile([128, NG, F], BF16)        # [h-in-group, g, f]
            # K^T per head parity, dead half zeroed so S can contract K=128
            KTe = persist.tile([128, NG, T], BF16)       # heads 2g   in rows 0-63
            KTo = persist.tile([128, NG, T], BF16)       # heads 2g+1 in rows 64-127
            Vsb = persist.tile([128, NT, HPC * HP1], BF16)
            bias_sb = persist.tile([128, 2 * NG], F32)
            bv_sb = persist.tile([128, HPC * H], BF16)
            vones_sb = persist.tile([128, 128], BF16)
            ones_bc_sb = persist.tile([128, HPC, 128], F16)
            nc.sync.dma_start(bias_sb[:], bqk[:])
            nc.sync.dma_start(bv_sb[:], bv_pad[:])
            nc.sync.dma_start(vones_sb[:], vones[:])
            nc.sync.dma_start(ones_bc_sb[:], ones_bc[:])
            act_warm = persist.tile([1, 1], F32)
            nc.scalar.activation(act_warm[:], bias_sb[0:1, 0:1],
                                 mybir.ActivationFunctionType.Exp)
            nc.vector.memset(KTe[64:128, :, :], 0.0)
            nc.vector.memset(KTo[0:64, :, :], 0.0)
            for nl in range(HPC):
                nc.vector.memset(Vsb[:, :, nl * HP1 + H], 1.0)

            with tc.tile_pool(name="p2", bufs=2) as p2:
                # prefetch the first mask block before phase-1 floods the DMAs
                maskT_re = maskT[:].rearrange("(a p) f -> p a f", p=128)
                masks = {}

                p1_cm = tc.tile_pool(name="p1", bufs=1)
                p1 = p1_cm.__enter__()
                ps1_cm = tc.tile_pool(name="ps1", bufs=2, space="PSUM")
                ps1 = ps1_cm.__enter__()
                sbuf_t = (QT, KTe, KTo, Vsb, bias_sb, bv_sb, vones_sb)
                wq_sb, fromT_sb, kv = _phase1_kv(nc, tc, p1, ps1, dict(
                    dram=(fromT, toT, wq, wk, wv),
                    sbuf=sbuf_t,
                ))

                masks[0] = p2.tile([128, NT, FB], BF16, tag="mask", name="mask")
                nc.sync.dma_start(masks[0][:], maskT_re[:, :, 0:FB])

                # ---- phase 2: attention ----
                with (
                    tc.tile_pool(name="p2e", bufs=3) as p2e,
                    tc.tile_pool(name="p2s", bufs=3) as p2s,
                    tc.tile_pool(name="p2r", bufs=2) as p2r,
                    tc.tile_pool(name="ps_s", bufs=2, space="PSUM") as ps_s,
                    tc.tile_pool(name="ps_c", bufs=1, space="PSUM") as ps_c,
                    tc.tile_pool(name="ps_b", bufs=1, space="PSUM") as ps_b,
                ):
                    GRPS = [2] * 8  # 16 t-tiles in ACT-sized groups
                    pending_norm = None
                    for j in range(NJ):
                        _emit_qt(nc, ps1, wq_sb, fromT_sb, QT, bias_sb, j)
                        mask_j = masks.pop(j)
                        if j + 1 < NJ:
                            masks[j + 1] = p2.tile([128, NT, FB], BF16,
                                                   tag="mask", name="mask")
                            nc.sync.dma_start(
                                masks[j + 1][:],
                                maskT_re[:, :, (j + 1) * FB:(j + 2) * FB],
                            )
                        sums_g = p2r.tile([128, FB], F32, tag="sums")
                        recip = p2r.tile([128, FB], F32, tag="recip")
                        recip_h = p2r.tile([128, FB], F16, tag="reciph")
                        nc.vector.memset(recip_h[:], 0.0)
                        ctx_keep = []
                        for n in range(HPC):
                            g, par = divmod(n, 2)
                            KT_ = KTe if par == 0 else KTo
                            ps_ctx = ps_c.tile([HP1, FB], F32, tag="ctx", name="ctx")
                            pend = None  # PV runs one group behind S/exp
                            t0 = 0
                            for qi, w in enumerate(GRPS):
                                if j == 0 and n == 0 and qi in (2, 4, 6):
                                    _emit_kv(nc, ps1, kv, sbuf_t, qi // 2)
                                ps_sq = ps_s.tile([128, w, FB], F32,
                                                  tag="sq", name="sq")
                                for i in range(w):
                                    nc.tensor.matmul(
                                        ps_sq[:, i, :],
                                        KT_[:, g, (t0 + i) * 128:(t0 + i + 1) * 128],
                                        QT[:, g, j * FB:(j + 1) * FB],
                                        start=True, stop=True,
                                    )
                                ex = p2e.tile([128, w, FB], BF16, tag="exp", name="exp")
                                nc.scalar.activation(
                                    ex[:], ps_sq[:],
                                    mybir.ActivationFunctionType.Exp,
                                    scale=0.125,
                                )
                                nc.vector.tensor_mul(
                                    ex[:], ex[:], mask_j[:, t0:t0 + w, :]
                                )
                                if pend is not None:
                                    pt0, pw, pex = pend
                                    for i in range(pw):
                                        ti = pt0 + i
                                        nc.tensor.matmul(
                                            ps_ctx[:],
                                            Vsb[:, ti, n * HP1:(n + 1) * HP1],
                                            pex[:, i, :],
                                            start=(ti == 0), stop=False,
                                        )
                                pend = (t0, w, ex)
                                t0 += w
                            pt0, pw, pex = pend
                            for i in range(pw):
                                ti = pt0 + i
                                nc.tensor.matmul(
                                    ps_ctx[:],
                                    Vsb[:, ti, n * HP1:(n + 1) * HP1],
                                    pex[:, i, :],
                                    start=False, stop=(ti == NT - 1),
                                )
                            ctx_sb = p2s.tile([HP1, FB], F32, tag="ctx_sb",
                                              name="ctx_sb", bufs=5)
                            nc.vector.tensor_copy(ctx_sb[:], ps_ctx[:])
                            # gather this head's sums row onto partition n
                            nc.gpsimd.dma_start(
                                sums_g[n:n + 1, :], ctx_sb[H:H + 1, :]
                            )
                            ctx_keep.append((n, ctx_sb))
                            if n == 0 and pending_norm is not None:
                                pending_norm()
                                pending_norm = None
                        # batched normalization for this j's 4 heads --
                        # deferred into the next j's stream so the long
                        # reciprocal doesn't block the DVE queue at the
                        # j boundary
                        def _norm(j=j, sums_g=sums_g, recip=recip,
                                  recip_h=recip_h, ctx_keep=list(ctx_keep)):
                            nc.vector.reciprocal(recip[0:HPC, :], sums_g[0:HPC, :])
                            nc.vector.tensor_copy(recip_h[0:HPC, :], recip[0:HPC, :])
                            for nn, ctx_sb in ctx_keep:
                                ps_bc = ps_b.tile([128, FB], F32, tag="bc",
                                                  name="ps_bc")
                                nc.tensor.matmul(
                                    ps_bc[:], ones_bc_sb[:, nn, :], recip_h[:],
                                    start=True, stop=True,
                                )
                                out_sb = p2s.tile([H, FB], F32, tag="out")
                                nc.vector.tensor_mul(
                                    out_sb[:], ctx_sb[0:H, :], ps_bc[0:H, :]
                                )
                                nc.gpsimd.dma_start(
                                    out_ctx[nn, :, j * FB:(j + 1) * FB],
                                    out_sb[:],
                                )
                        pending_norm = _norm
                    pending_norm()
                p1_cm.__exit__(None, None, None)
                ps1_cm.__exit__(None, None, None)

    nc.compile()
    return nc


_compiled = None


def _get_compiled():
    global _compiled
    if _compiled is None:
        _compiled = _program()
    return _compiled


def make_in_maps(from_tensor, to_tensor, attention_mask, wq, bq, wk, bk, wv, bv):
    bf = ml_dtypes.bfloat16
    from_tensor = np.asarray(from_tensor, dtype=np.float32)
    to_tensor = np.asarray(to_tensor, dtype=np.float32)
    attention_mask = np.asarray(attention_mask)
    wq = np.asarray(wq, dtype=np.float32)
    wk = np.asarray(wk, dtype=np.float32)
    wv = np.asarray(wv, dtype=np.float32)
    bq = np.asarray(bq, dtype=np.float32)
    bk = np.asarray(bk, dtype=np.float32)
    bv = np.asarray(bv, dtype=np.float32)

    fromT_b = [np.ascontiguousarray(from_tensor[b].T).astype(bf) for b in range(B)]
    toT_b = [np.ascontiguousarray(to_tensor[b].T).astype(bf) for b in range(B)]
    maskT_b = [attention_mask[b].T.astype(bf) for b in range(B)]
    vones_arr = np.zeros((128, 128), dtype=bf)
    vones_arr[0, :] = 1.0
    ones_bc_arr = np.zeros((128, HPC, 128), dtype=np.float16)
    for nn in range(HPC):
        ones_bc_arr[nn, nn, :] = 1.0

    in_maps = []
    for c in range(NCORES):
        b, hb = divmod(c, NCORES // B)
        hs = hb * HPC
        bq_dev = bq[hs:hs + HPC].reshape(NG, 128).T
        bk_dev = bk[hs:hs + HPC].reshape(NG, 128).T
        bv_pad = np.zeros((128, HPC * H), dtype=bf)
        bv_pad[0, :] = bv[hs:hs + HPC].reshape(HPC * H)
        in_maps.append(
            dict(
                fromT=fromT_b[b],
                toT=toT_b[b],
                maskT=maskT_b[b],
                wq=wq[:, hs:hs + HPC, :].reshape(D, HPC * H).astype(bf),
                wk=wk[:, hs:hs + HPC, :].reshape(D, HPC * H).astype(bf),
                wv=wv[:, hs:hs + HPC, :].reshape(D, HPC * H).astype(bf),
                bqk=np.ascontiguousarray(
                    np.concatenate([bq_dev, bk_dev], axis=1), dtype=np.float32
                ),
                bv_pad=bv_pad,
                vones=vones_arr,
                ones_bc=ones_bc_arr,
            )
        )
    return in_maps


def gather_output(results):
    out = np.empty((B, F, N, H), dtype=np.float32)
    for c in range(NCORES):
        b, hb = divmod(c, NCORES // B)
        hs = hb * HPC
        ctx = results[c]["out_ctx"]  # [HPC, H, F]
        out[b, :, hs:hs + HPC, :] = ctx.transpose(2, 0, 1)
    return out


def run_sharded(inputs, **run_kwargs):
    """Run the SPMD kernel; returns (output, BassKernelResults)."""
    nc = _get_compiled()
    in_maps = make_in_maps(**inputs)
    res = run_bass_kernel_spmd(nc, in_maps, list(range(NCORES)), **run_kwargs)
    return gather_output(res.results), res


def kernel(**inputs):
    out, _ = run_sharded(inputs)
    return out


# revision 30
# speedup vs baseline: 1.0357x; 1.0357x over previous
"""Multi-head attention (QKV projection + masked softmax + PV) on 8 TRN2
NeuronCores.

Sharding: data-parallel over batch (B=2 -> 2 groups of 4 cores), tensor
parallel over heads (16 heads -> 4 heads per core). Each core computes full
F x T attention for its 4 heads.

Per-core device algorithm (kept transposed so the softmax reduction lands on
the TensorE contraction dim; all matmuls bf16/fp16, PSUM fp32):
  Q^T[h,f] = wq^T @ from^T        K^T[h,t] = wk^T @ to^T
  V[t,hh]  = to^T^T @ wv   (+ a ones column per head for the softmax sums)
  S^T[t,f] = K^T(stationary, zero-padded to K=128) x Q^T(moving)
  E = exp(S^T/8) (ScalarE, PSUM->SBUF, bf16);  E *= mask^T (bf16)
  ctx^T[h,f] (+ sums[f] via the ones column) = sum_t V x E
  out = ctx^T * (1/sums)   reciprocals batched 4-per-j; broadcast down the
                           h-partitions via a selector-row ones matmul
Every matmul keeps tile_size (128,128) -- K=1 products are zero-padded to
K=128 via selector rows -- so the PE array never drains for a mode switch,
and PV is emitted one quad behind S so the PE stream never stalls long
enough for the HAM clock gate to re-throttle.

Host does the cheap layout work: pre-transposes from/to/mask (bf16), slices
weights per head group, transposes the [4,64,2048] per-core results back into
[B,F,N,H].
"""

import os
import sys

for _p in ("/opt/trn_rl_repo",):
    if os.path.isdir(_p) and _p not in sys.path:
        sys.path.insert(0, _p)

import numpy as np
import ml_dtypes

import concourse.tile as tile
from concourse import bacc, mybir
from concourse.bass_utils import run_bass_kernel_spmd

B, F, T, D, N, H = 2, 2048, 2048, 1024, 16, 64
NCORES = 8
HPC = N // (NCORES // B)  # heads per core = 4
NG = HPC // 2             # 128-partition head groups (2 heads each) = 2
FB = 512                  # f-block (psum bank width in fp32)
NJ = F // FB              # 4
NT = T // 128             # 16 t-tiles
NK = D // 128             # 8 contraction tiles
HP1 = H + 1               # head V columns incl. the ones column

F32 = mybir.dt.float32
F16 = mybir.dt.float16
BF16 = mybir.dt.bfloat16


def _phase1_kv(nc, tc, p1, ps1, tensors):
    """Load inputs, compute K^T (parity-split) and V, both per t-block as the
    toT chunks land. Q^T is emitted later, interleaved with the attention
    j-loop, so ScalarE starts exp'ing as soon as K^T and Q^T(j0) exist."""
    (fromT, toT, wq, wk, wv) = tensors["dram"]
    (QT, KTe, KTo, Vsb, bias_sb, bv_sb, vones_sb) = tensors["sbuf"]
    toT_sb = p1.tile([128, NK, T], BF16)
    fromT_sb = p1.tile([128, NK, F], BF16)
    wq_sb = p1.tile([128, NK, HPC * H], BF16)
    wk_sb = p1.tile([128, NK, HPC * H], BF16)
    wv_sb = p1.tile([128, NK, HPC * H], BF16)
    # critical path (HW-DGE queue): wk + toT t-block0 + wv + wq
    for k in range(NK):
        nc.sync.dma_start(wk_sb[:, k, :], wk[k * 128:(k + 1) * 128, :])
        nc.sync.dma_start(toT_sb[:, k, 0:FB], toT[k * 128:(k + 1) * 128, 0:FB])
    nc.sync.dma_start(wv_sb[:], wv[:].rearrange("(k p) m -> p k m", p=128))
    for k in range(NK):
        nc.sync.dma_start(wq_sb[:, k, :], wq[k * 128:(k + 1) * 128, :])
    # SW-DGE queue: fromT (for Q^T at the start of the attention loop),
    # then the remaining toT t-blocks
    for k in range(NK):
        nc.gpsimd.dma_start(fromT_sb[:, k, :], fromT[k * 128:(k + 1) * 128, :])
    for tb in range(1, NJ):
        for k in range(NK):
            nc.gpsimd.dma_start(
                toT_sb[:, k, tb * FB:(tb + 1) * FB],
                toT[k * 128:(k + 1) * 128, tb * FB:(tb + 1) * FB],
            )

    kv = dict(toT_sb=toT_sb, wk_sb=wk_sb, wv_sb=wv_sb)
    _emit_kv(nc, ps1, kv, tensors["sbuf"], 0)
    return wq_sb, fromT_sb, kv


def _emit_kv(nc, ps1, kv, sbuf, tb):
    (QT, KTe, KTo, Vsb, bias_sb, bv_sb, vones_sb) = sbuf
    toT_sb, wk_sb, wv_sb = kv["toT_sb"], kv["wk_sb"], kv["wv_sb"]
    for g in range(NG):
        ps_qk = ps1.tile([128, FB], F32, tag="qk")
        for k in range(NK):
            nc.tensor.matmul(
                ps_qk[:],
                wk_sb[:, k, g * 128:(g + 1) * 128],
                toT_sb[:, k, tb * FB:(tb + 1) * FB],
                start=(k == 0),
                stop=(k == NK - 1),
            )
        nc.vector.tensor_scalar_add(
            KTe[0:64, g, tb * FB:(tb + 1) * FB],
            ps_qk[0:64, :],
            bias_sb[0:64, NG + g:NG + g + 1],
        )
        nc.vector.tensor_scalar_add(
            KTo[64:128, g, tb * FB:(tb + 1) * FB],
            ps_qk[64:128, :],
            bias_sb[64:128, NG + g:NG + g + 1],
        )
    for ti in range(tb * 4, tb * 4 + 4):
        ps_v = ps1.tile([128, HPC * H], F32, tag="qk", name="ps_v")
        for k in range(NK):
            nc.tensor.matmul(
                ps_v[:],
                toT_sb[:, k, ti * 128:(ti + 1) * 128],
                wv_sb[:, k, :],
                start=(k == 0),
                stop=False,
            )
        nc.tensor.matmul(ps_v[:], vones_sb[:], bv_sb[:], start=False, stop=True)
        for nl in range(HPC):
            nc.vector.tensor_copy(
                Vsb[:, ti, nl * HP1:nl * HP1 + H],
                ps_v[:, nl * H:(nl + 1) * H],
            )


def _emit_qt(nc, ps1, wq_sb, fromT_sb, QT, bias_sb, j):
    for g in range(NG):
        ps_qk = ps1.tile([128, FB], F32, tag="qk")
        for k in range(NK):
            nc.tensor.matmul(
                ps_qk[:],
                wq_sb[:, k, g * 128:(g + 1) * 128],
                fromT_sb[:, k, j * FB:(j + 1) * FB],
                start=(k == 0),
                stop=(k == NK - 1),
            )
        nc.vector.tensor_scalar_add(
            QT[:, g, j * FB:(j + 1) * FB],
            ps_qk[:],
            bias_sb[:, g:g + 1],
        )


def _program():
    nc = bacc.Bacc(None, target_bir_lowering=False)
    fromT = nc.declare_dram_parameter("fromT", [D, F], BF16, isOutput=False)
    toT = nc.declare_dram_parameter("toT", [D, T], BF16, isOutput=False)
    maskT = nc.declare_dram_parameter("maskT", [T, F], BF16, isOutput=False)
    wq = nc.declare_dram_parameter("wq", [D, HPC * H], BF16, isOutput=False)
    wk = nc.declare_dram_parameter("wk", [D, HPC * H], BF16, isOutput=False)
    wv = nc.declare_dram_parameter("wv", [D, HPC * H], BF16, isOutput=False)
    bqk = nc.declare_dram_parameter("bqk", [128, 2 * NG], F32, isOutput=False)
    # bv padded to K=128 (row 0 = bv, rest zero) for a mode-switch-free matmul
    bv_pad = nc.declare_dram_parameter("bv_pad", [128, HPC * H], BF16, isOutput=False)
    # all-ones row 0 (rest zero): stationary operand of the bv matmul
    vones = nc.declare_dram_parameter("vones", [128, 128], BF16, isOutput=False)
    # selector blocks: ones_bc[k, nn, m] = (k == nn), broadcast matmul lhsT
    ones_bc = nc.declare_dram_parameter("ones_bc", [128, HPC, 128], F16, isOutput=False)
    out_ctx = nc.declare_dram_parameter("out_ctx", [HPC, H, F], F32, isOutput=True)

    with tile.TileContext(nc) as tc:
        with tc.tile_pool(name="persist", bufs=1) as persist:
            QT = persist.tile([128, NG, F], BF16)        # [h-in-group, g, f]
            # K^T per head parity, dead half zeroed so S can contract K=128
            KTe = persist.tile([128, NG, T], BF16)       # heads 2g   in rows 0-63
            KTo = persist.tile([128, NG, T], BF16)       # heads 2g+1 in rows 64-127
            Vsb = persist.tile([128, NT, HPC * HP1], BF16)
            bias_sb = persist.tile([128, 2 * NG], F32)
            bv_sb = persist.tile([128, HPC * H], BF16)
            vones_sb = persist.tile([128, 128], BF16)
            ones_bc_sb = persist.tile([128, HPC, 128], F16)
            nc.sync.dma_start(bias_sb[:], bqk[:])
            nc.sync.dma_start(bv_sb[:], bv_pad[:])
            nc.sync.dma_start(vones_sb[:], vones[:])
            nc.sync.dma_start(ones_bc_sb[:], ones_bc[:])
            act_warm = persist.tile([1, 1], F32)
            nc.scalar.activation(act_warm[:], bias_sb[0:1, 0:1],
                                 mybir.ActivationFunctionType.Exp)
            nc.vector.memset(KTe[64:128, :, :], 0.0)
            nc.vector.memset(KTo[0:64, :, :], 0.0)
            for nl in range(HPC):
                nc.vector.memset(Vsb[:, :, nl * HP1 + H], 1.0)

            with tc.tile_pool(name="p2", bufs=2) as p2:
                # prefetch the first mask block before phase-1 floods the DMAs
                maskT_re = maskT[:].rearrange("(a p) f -> p a f", p=128)
                masks = {}

                p1_cm = tc.tile_pool(name="p1", bufs=1)
                p1 = p1_cm.__enter__()
                ps1_cm = tc.tile_pool(name="ps1", bufs=2, space="PSUM")
                ps1 = ps1_cm.__enter__()
                sbuf_t = (QT, KTe, KTo, Vsb, bias_sb, bv_sb, vones_sb)
                wq_sb, fromT_sb, kv = _phase1_kv(nc, tc, p1, ps1, dict(
                    dram=(fromT, toT, wq, wk, wv),
                    sbuf=sbuf_t,
                ))

                masks[0] = p2.tile([128, NT, FB], BF16, tag="mask", name="mask")
                nc.sync.dma_start(masks[0][:], maskT_re[:, :, 0:FB])

                # ---- phase 2: attention ----
                with (
                    tc.tile_pool(name="p2e", bufs=3) as p2e,
                    tc.tile_pool(name="p2s", bufs=3) as p2s,
                    tc.tile_pool(name="p2r", bufs=2) as p2r,
                    tc.tile_pool(name="ps_s", bufs=2, space="PSUM") as ps_s,
                    tc.tile_pool(name="ps_c", bufs=1, space="PSUM") as ps_c,
                    tc.tile_pool(name="ps_b", bufs=1, space="PSUM") as ps_b,
                ):
                    GRPS = [2] * 8  # 16 t-tiles in ACT-sized groups
                    pending_norm = None
                    for j in range(NJ):
                        _emit_qt(nc, ps1, wq_sb, fromT_sb, QT, bias_sb, j)
                        mask_j = masks.pop(j)
                        if j + 1 < NJ:
                            masks[j + 1] = p2.tile([128, NT, FB], BF16,
                                                   tag="mask", name="mask")
                            nc.sync.dma_start(
                                masks[j + 1][:],
                                maskT_re[:, :, (j + 1) * FB:(j + 2) * FB],
                            )
                        sums_g = p2r.tile([128, FB], F32, tag="sums")
                        recip = p2r.tile([128, FB], F32, tag="recip")
                        recip_h = p2r.tile([128, FB], F16, tag="reciph")
                        nc.vector.memset(recip_h[:], 0.0)
                        ctx_keep = []
                        for n in range(HPC):
                            g, par = divmod(n, 2)
                            KT_ = KTe if par == 0 else KTo
                            ps_ctx = ps_c.tile([HP1, FB], F32, tag="ctx", name="ctx")
                            pend = None  # PV runs one group behind S/exp
                            t0 = 0
                            for qi, w in enumerate(GRPS):
                                if j == 0 and n == 0 and qi in (2, 4, 6):
                                    _emit_kv(nc, ps1, kv, sbuf_t, qi // 2)
                                ps_sq = ps_s.tile([128, w, FB], F32,
                                                  tag="sq", name="sq")
                                for i in range(w):
                                    nc.tensor.matmul(
                                        ps_sq[:, i, :],
                                        KT_[:, g, (t0 + i) * 128:(t0 + i + 1) * 128],
                                        QT[:, g, j * FB:(j + 1) * FB],
                                        start=True, stop=True,
                                    )
                                ex = p2e.tile([128, w, FB], BF16, tag="exp", name="exp")
                                nc.scalar.activation(
                                    ex[:], ps_sq[:],
                                    mybir.ActivationFunctionType.Exp,
                                    scale=0.125,
                                )
                                nc.vector.tensor_mul(
                                    ex[:], ex[:], mask_j[:, t0:t0 + w, :]
                                )
                                if pend is not None:
                                    pt0, pw, pex = pend
                                    for i in range(pw):
                                        ti = pt0 + i
                                        nc.tensor.matmul(
                                            ps_ctx[:],
                                            Vsb[:, ti, n * HP1:(n + 1) * HP1],
                                            pex[:, i, :],
                                            start=(ti == 0), stop=False,
                                        )
                                pend = (t0, w, ex)
                                t0 += w
                            pt0, pw, pex = pend
                            for i in range(pw):
                                ti = pt0 + i
                                nc.tensor.matmul(
                                    ps_ctx[:],
                                    Vsb[:, ti, n * HP1:(n + 1) * HP1],
                                    pex[:, i, :],
                                    start=False, stop=(ti == NT - 1),
                                )
                            ctx_sb = p2s.tile([HP1, FB], F32, tag="ctx_sb",
                                              name="ctx_sb", bufs=5)
                            nc.vector.tensor_copy(ctx_sb[:], ps_ctx[:])
                            # gather this head's sums row onto partition n
                            nc.gpsimd.dma_start(
                                sums_g[n:n + 1, :], ctx_sb[H:H + 1, :]
                            )
                            ctx_keep.append((n, ctx_sb))
                            if n in (0, 1) and pending_norm is not None:
                                pending_norm(n)
                                if n == 1:
                                    pending_norm = None
                        # batched normalization for this j's 4 heads --
                        # deferred into the next j's stream so the long
                        # reciprocal doesn't block the DVE queue at the
                        # j boundary
                        def _norm(step, j=j, sums_g=sums_g, recip=recip,
                                  recip_h=recip_h, ctx_keep=list(ctx_keep)):
                            CH = FB // 4
                            if step == 0:
                                for c in range(2):
                                    nc.vector.reciprocal(
                                        recip[0:HPC, c * CH:(c + 1) * CH],
                                        sums_g[0:HPC, c * CH:(c + 1) * CH],
                                    )
                                return
                            for c in range(2, 4):
                                nc.vector.reciprocal(
                                    recip[0:HPC, c * CH:(c + 1) * CH],
                                    sums_g[0:HPC, c * CH:(c + 1) * CH],
                                )
                            nc.vector.tensor_copy(recip_h[0:HPC, :], recip[0:HPC, :])
                            for nn, ctx_sb in ctx_keep:
                                ps_bc = ps_b.tile([128, FB], F32, tag="bc",
                                                  name="ps_bc")
                                nc.tensor.matmul(
                                    ps_bc[:], ones_bc_sb[:, nn, :], recip_h[:],
                                    start=True, stop=True,
                                )
                                out_sb = p2s.tile([H, FB], F32, tag="out")
                                nc.vector.tensor_mul(
                                    out_sb[:], ctx_sb[0:H, :], ps_bc[0:H, :]
                                )
                                nc.gpsimd.dma_start(
                                    out_ctx[nn, :, j * FB:(j + 1) * FB],
                                    out_sb[:],
                                )
                        pending_norm = _norm
                    pending_norm(0)
                    pending_norm(1)
                p1_cm.__exit__(None, None, None)
                ps1_cm.__exit__(None, None, None)

    nc.compile()
    return nc


_compiled = None


def _get_compiled():
    global _compiled
    if _compiled is None:
        _compiled = _program()
    return _compiled


def make_in_maps(from_tensor, to_tensor, attention_mask, wq, bq, wk, bk, wv, bv):
    bf = ml_dtypes.bfloat16
    from_tensor = np.asarray(from_tensor, dtype=np.float32)
    to_tensor = np.asarray(to_tensor, dtype=np.float32)
    attention_mask = np.asarray(attention_mask)
    wq = np.asarray(wq, dtype=np.float32)
    wk = np.asarray(wk, dtype=np.float32)
    wv = np.asarray(wv, dtype=np.float32)
    bq = np.asarray(bq, dtype=np.float32)
    bk = np.asarray(bk, dtype=np.float32)
    bv = np.asarray(bv, dtype=np.float32)

    fromT_b = [np.ascontiguousarray(from_tensor[b].T).astype(bf) for b in range(B)]
    toT_b = [np.ascontiguousarray(to_tensor[b].T).astype(bf) for b in range(B)]
    maskT_b = [attention_mask[b].T.astype(bf) for b in range(B)]
    vones_arr = np.zeros((128, 128), dtype=bf)
    vones_arr[0, :] = 1.0
    ones_bc_arr = np.zeros((128, HPC, 128), dtype=np.float16)
    for nn in range(HPC):
        ones_bc_arr[nn, nn, :] = 1.0

    in_maps = []
    for c in range(NCORES):
        b, hb = divmod(c, NCORES // B)
        hs = hb * HPC
        bq_dev = bq[hs:hs + HPC].reshape(NG, 128).T
        bk_dev = bk[hs:hs + HPC].reshape(NG, 128).T
        bv_pad = np.zeros((128, HPC * H), dtype=bf)
        bv_pad[0, :] = bv[hs:hs + HPC].reshape(HPC * H)
        in_maps.append(
            dict(
                fromT=fromT_b[b],
                toT=toT_b[b],
                maskT=maskT_b[b],
                wq=wq[:, hs:hs + HPC, :].reshape(D, HPC * H).astype(bf),
                wk=wk[:, hs:hs + HPC, :].reshape(D, HPC * H).astype(bf),
                wv=wv[:, hs:hs + HPC, :].reshape(D, HPC * H).astype(bf),
                bqk=np.ascontiguousarray(
                    np.concatenate([bq_dev, bk_dev], axis=1), dtype=np.float32
                ),
                bv_pad=bv_pad,
                vones=vones_arr,
                ones_bc=ones_bc_arr,
            )
        )
    return in_maps


def gather_output(results):
    out = np.empty((B, F, N, H), dtype=np.float32)
    for c in range(NCORES):
        b, hb = divmod(c, NCORES // B)
        hs = hb * HPC
        ctx = results[c]["out_ctx"]  # [HPC, H, F]
        out[b, :, hs:hs + HPC, :] = ctx.transpose(2, 0, 1)
    return out


def run_sharded(inputs, **run_kwargs):
    """Run the SPMD kernel; returns (output, BassKernelResults)."""
    nc = _get_compiled()
    in_maps = make_in_maps(**inputs)
    res = run_bass_kernel_spmd(nc, in_maps, list(range(NCORES)), **run_kwargs)
    return gather_output(res.results), res


def kernel(**inputs):
    out, _ = run_sharded(inputs)
    return out


# revision 31
# speedup vs baseline: 1.0442x; 1.0082x over previous
"""Multi-head attention (QKV projection + masked softmax + PV) on 8 TRN2
NeuronCores.

Sharding: data-parallel over batch (B=2 -> 2 groups of 4 cores), tensor
parallel over heads (16 heads -> 4 heads per core). Each core computes full
F x T attention for its 4 heads.

Per-core device algorithm (kept transposed so the softmax reduction lands on
the TensorE contraction dim; all matmuls bf16/fp16, PSUM fp32):
  Q^T[h,f] = wq^T @ from^T        K^T[h,t] = wk^T @ to^T
  V[t,hh]  = to^T^T @ wv   (+ a ones column per head for the softmax sums)
  S^T[t,f] = K^T(stationary, zero-padded to K=128) x Q^T(moving)
  E = exp(S^T/8) (ScalarE, PSUM->SBUF, bf16);  E *= mask^T (bf16)
  ctx^T[h,f] (+ sums[f] via the ones column) = sum_t V x E
  out = ctx^T * (1/sums)   reciprocals batched 4-per-j; broadcast down the
                           h-partitions via a selector-row ones matmul
Every matmul keeps tile_size (128,128) -- K=1 products are zero-padded to
K=128 via selector rows -- so the PE array never drains for a mode switch,
and PV is emitted one quad behind S so the PE stream never stalls long
enough for the HAM clock gate to re-throttle.

Host does the cheap layout work: pre-transposes from/to/mask (bf16), slices
weights per head group, transposes the [4,64,2048] per-core results back into
[B,F,N,H].
"""

import os
import sys

for _p in ("/opt/trn_rl_repo",):
    if os.path.isdir(_p) and _p not in sys.path:
        sys.path.insert(0, _p)

import numpy as np
import ml_dtypes

import concourse.tile as tile
from concourse import bacc, mybir
from concourse.bass_utils import run_bass_kernel_spmd

B, F, T, D, N, H = 2, 2048, 2048, 1024, 16, 64
NCORES = 8
HPC = N // (NCORES // B)  # heads per core = 4
NG = HPC // 2             # 128-partition head groups (2 heads each) = 2
FB = 512                  # f-block (psum bank width in fp32)
NJ = F // FB              # 4
NT = T // 128             # 16 t-tiles
NK = D // 128             # 8 contraction tiles
HP1 = H + 1               # head V columns incl. the ones column

# (j0, head0) slot -> K/V pieces to interleave, deadline-aware:
# K(tb) must land before S-group 2*tb; V(ti) before the lagged PV that reads it
KV_SLOTS = {
    0: (("k", 1, 0), ("k", 1, 1)),
    1: (("v", 4, 0), ("v", 5, 0)),
    2: (("v", 6, 0), ("v", 7, 0)),
    3: (("k", 2, 0), ("k", 2, 1)),
    4: (("v", 8, 0), ("v", 9, 0)),
    5: (("k", 3, 0), ("k", 3, 1)),
    6: (("v", 10, 0), ("v", 11, 0)),
    7: (("v", 12, 0), ("v", 13, 0), ("v", 14, 0), ("v", 15, 0)),
}

F32 = mybir.dt.float32
F16 = mybir.dt.float16
BF16 = mybir.dt.bfloat16


def _phase1_kv(nc, tc, p1, ps1, tensors):
    """Load inputs, compute K^T (parity-split) and V, both per t-block as the
    toT chunks land. Q^T is emitted later, interleaved with the attention
    j-loop, so ScalarE starts exp'ing as soon as K^T and Q^T(j0) exist."""
    (fromT, toT, wq, wk, wv) = tensors["dram"]
    (QT, KTe, KTo, Vsb, bias_sb, bv_sb, vones_sb) = tensors["sbuf"]
    toT_sb = p1.tile([128, NK, T], BF16)
    fromT_sb = p1.tile([128, NK, F], BF16)
    wq_sb = p1.tile([128, NK, HPC * H], BF16)
    wk_sb = p1.tile([128, NK, HPC * H], BF16)
    wv_sb = p1.tile([128, NK, HPC * H], BF16)
    # critical path (HW-DGE queue): wk + toT t-block0 + wv + wq
    for k in range(NK):
        nc.sync.dma_start(wk_sb[:, k, :], wk[k * 128:(k + 1) * 128, :])
        nc.sync.dma_start(toT_sb[:, k, 0:FB], toT[k * 128:(k + 1) * 128, 0:FB])
    nc.sync.dma_start(wv_sb[:], wv[:].rearrange("(k p) m -> p k m", p=128))
    for k in range(NK):
        nc.sync.dma_start(wq_sb[:, k, :], wq[k * 128:(k + 1) * 128, :])
    # SW-DGE queue: fromT (for Q^T at the start of the attention loop),
    # then the remaining toT t-blocks
    for k in range(NK):
        nc.gpsimd.dma_start(
            fromT_sb[:, k, 0:FB], fromT[k * 128:(k + 1) * 128, 0:FB]
        )
    for tb in range(1, NJ):
        for k in range(NK):
            nc.gpsimd.dma_start(
                toT_sb[:, k, tb * FB:(tb + 1) * FB],
                toT[k * 128:(k + 1) * 128, tb * FB:(tb + 1) * FB],
            )
    for jb in range(1, NJ):
        for k in range(NK):
            nc.gpsimd.dma_start(
                fromT_sb[:, k, jb * FB:(jb + 1) * FB],
                fromT[k * 128:(k + 1) * 128, jb * FB:(jb + 1) * FB],
            )

    kv = dict(toT_sb=toT_sb, wk_sb=wk_sb, wv_sb=wv_sb)
    for g in range(NG):
        _emit_k_piece(nc, ps1, kv, tensors["sbuf"], 0, g)
    for ti in range(4):
        _emit_v_piece(nc, ps1, kv, tensors["sbuf"], ti)
    return wq_sb, fromT_sb, kv


def _emit_k_piece(nc, ps1, kv, sbuf, tb, g):
    (QT, KTe, KTo, Vsb, bias_sb, bv_sb, vones_sb) = sbuf
    toT_sb, wk_sb = kv["toT_sb"], kv["wk_sb"]
    ps_qk = ps1.tile([128, FB], F32, tag="qk", name="ps_qk")
    for k in range(NK):
        nc.tensor.matmul(
            ps_qk[:],
            wk_sb[:, k, g * 128:(g + 1) * 128],
            toT_sb[:, k, tb * FB:(tb + 1) * FB],
            start=(k == 0),
            stop=(k == NK - 1),
        )
    nc.vector.tensor_scalar_add(
        KTe[0:64, g, tb * FB:(tb + 1) * FB],
        ps_qk[0:64, :],
        bias_sb[0:64, NG + g:NG + g + 1],
    )
    nc.vector.tensor_scalar_add(
        KTo[64:128, g, tb * FB:(tb + 1) * FB],
        ps_qk[64:128, :],
        bias_sb[64:128, NG + g:NG + g + 1],
    )


def _emit_v_piece(nc, ps1, kv, sbuf, ti):
    (QT, KTe, KTo, Vsb, bias_sb, bv_sb, vones_sb) = sbuf
    toT_sb, wv_sb = kv["toT_sb"], kv["wv_sb"]
    ps_v = ps1.tile([128, HPC * H], F32, tag="qk", name="ps_v")
    for k in range(NK):
        nc.tensor.matmul(
            ps_v[:],
            toT_sb[:, k, ti * 128:(ti + 1) * 128],
            wv_sb[:, k, :],
            start=(k == 0),
            stop=False,
        )
    nc.tensor.matmul(ps_v[:], vones_sb[:], bv_sb[:], start=False, stop=True)
    for nl in range(HPC):
        nc.vector.tensor_copy(
            Vsb[:, ti, nl * HP1:nl * HP1 + H],
            ps_v[:, nl * H:(nl + 1) * H],
        )


def _emit_qt(nc, ps1, wq_sb, fromT_sb, QT, bias_sb, j):
    for g in range(NG):
        ps_qk = ps1.tile([128, FB], F32, tag="qk")
        for k in range(NK):
            nc.tensor.matmul(
                ps_qk[:],
                wq_sb[:, k, g * 128:(g + 1) * 128],
                fromT_sb[:, k, j * FB:(j + 1) * FB],
                start=(k == 0),
                stop=(k == NK - 1),
            )
        nc.vector.tensor_scalar_add(
            QT[:, g, j * FB:(j + 1) * FB],
            ps_qk[:],
            bias_sb[:, g:g + 1],
        )


def _program():
    nc = bacc.Bacc(None, target_bir_lowering=False)
    fromT = nc.declare_dram_parameter("fromT", [D, F], BF16, isOutput=False)
    toT = nc.declare_dram_parameter("toT", [D, T], BF16, isOutput=False)
    maskT = nc.declare_dram_parameter("maskT", [T, F], BF16, isOutput=False)
    wq = nc.declare_dram_parameter("wq", [D, HPC * H], BF16, isOutput=False)
    wk = nc.declare_dram_parameter("wk", [D, HPC * H], BF16, isOutput=False)
    wv = nc.declare_dram_parameter("wv", [D, HPC * H], BF16, isOutput=False)
    bqk = nc.declare_dram_parameter("bqk", [128, 2 * NG], F32, isOutput=False)
    # bv padded to K=128 (row 0 = bv, rest zero) for a mode-switch-free matmul
    bv_pad = nc.declare_dram_parameter("bv_pad", [128, HPC * H], BF16, isOutput=False)
    # all-ones row 0 (rest zero): stationary operand of the bv matmul
    vones = nc.declare_dram_parameter("vones", [128, 128], BF16, isOutput=False)
    # selector blocks: ones_bc[k, nn, m] = (k == nn), broadcast matmul lhsT
    ones_bc = nc.declare_dram_parameter("ones_bc", [128, HPC, 128], F16, isOutput=False)
    out_ctx = nc.declare_dram_parameter("out_ctx", [HPC, H, F], F32, isOutput=True)

    with tile.TileContext(nc) as tc:
        with tc.tile_pool(name="persist", bufs=1) as persist:
            QT = persist.tile([128, NG, F], BF16)        # [h-in-group, g, f]
            # K^T per head parity, dead half zeroed so S can contract K=128
            KTe = persist.tile([128, NG, T], BF16)       # heads 2g   in rows 0-63
            KTo = persist.tile([128, NG, T], BF16)       # heads 2g+1 in rows 64-127
            Vsb = persist.tile([128, NT, HPC * HP1], BF16)
            bias_sb = persist.tile([128, 2 * NG], F32)
            bv_sb = persist.tile([128, HPC * H], BF16)
            vones_sb = persist.tile([128, 128], BF16)
            ones_bc_sb = persist.tile([128, HPC, 128], F16)
            nc.sync.dma_start(bias_sb[:], bqk[:])
            nc.sync.dma_start(bv_sb[:], bv_pad[:])
            nc.sync.dma_start(vones_sb[:], vones[:])
            nc.sync.dma_start(ones_bc_sb[:], ones_bc[:])
            act_warm = persist.tile([1, 1], F32)
            nc.scalar.activation(act_warm[:], bias_sb[0:1, 0:1],
                                 mybir.ActivationFunctionType.Exp)
            nc.vector.memset(KTe[64:128, :, :], 0.0)
            nc.vector.memset(KTo[0:64, :, :], 0.0)
            for nl in range(HPC):
                nc.vector.memset(Vsb[:, :, nl * HP1 + H], 1.0)

            with tc.tile_pool(name="p2", bufs=2) as p2:
                # prefetch the first mask block before phase-1 floods the DMAs
                maskT_re = maskT[:].rearrange("(a p) f -> p a f", p=128)
                masks = {}

                p1_cm = tc.tile_pool(name="p1", bufs=1)
                p1 = p1_cm.__enter__()
                ps1_cm = tc.tile_pool(name="ps1", bufs=2, space="PSUM")
                ps1 = ps1_cm.__enter__()
                sbuf_t = (QT, KTe, KTo, Vsb, bias_sb, bv_sb, vones_sb)
                wq_sb, fromT_sb, kv = _phase1_kv(nc, tc, p1, ps1, dict(
                    dram=(fromT, toT, wq, wk, wv),
                    sbuf=sbuf_t,
                ))

                masks[0] = p2.tile([128, NT, FB], BF16, tag="mask", name="mask")
                nc.sync.dma_start(masks[0][:], maskT_re[:, :, 0:FB])

                # ---- phase 2: attention ----
                with (
                    tc.tile_pool(name="p2e", bufs=3) as p2e,
                    tc.tile_pool(name="p2s", bufs=3) as p2s,
                    tc.tile_pool(name="p2r", bufs=2) as p2r,
                    tc.tile_pool(name="ps_s", bufs=2, space="PSUM") as ps_s,
                    tc.tile_pool(name="ps_c", bufs=1, space="PSUM") as ps_c,
                    tc.tile_pool(name="ps_b", bufs=1, space="PSUM") as ps_b,
                ):
                    GRPS = [2] * 8  # 16 t-tiles in ACT-sized groups
                    pending_norm = None
                    for j in range(NJ):
                        _emit_qt(nc, ps1, wq_sb, fromT_sb, QT, bias_sb, j)
                        mask_j = masks.pop(j)
                        if j + 1 < NJ:
                            masks[j + 1] = p2.tile([128, NT, FB], BF16,
                                                   tag="mask", name="mask")
                            nc.sync.dma_start(
                                masks[j + 1][:],
                                maskT_re[:, :, (j + 1) * FB:(j + 2) * FB],
                            )
                        sums_g = p2r.tile([128, FB], F32, tag="sums")
                        recip = p2r.tile([128, FB], F32, tag="recip")
                        recip_h = p2r.tile([128, FB], F16, tag="reciph")
                        nc.vector.memset(recip_h[:], 0.0)
                        ctx_keep = []
                        for n in range(HPC):
                            g, par = divmod(n, 2)
                            KT_ = KTe if par == 0 else KTo
                            ps_ctx = ps_c.tile([HP1, FB], F32, tag="ctx", name="ctx")
                            pend = None  # PV runs one group behind S/exp
                            t0 = 0
                            for qi, w in enumerate(GRPS):
                                if j == 0 and n == 0:
                                    for kind, a, b in KV_SLOTS.get(qi, ()):
                                        if kind == "k":
                                            _emit_k_piece(nc, ps1, kv, sbuf_t, a, b)
                                        else:
                                            _emit_v_piece(nc, ps1, kv, sbuf_t, a)
                                ps_sq = ps_s.tile([128, w, FB], F32,
                                                  tag="sq", name="sq")
                                for i in range(w):
                                    nc.tensor.matmul(
                                        ps_sq[:, i, :],
                                        KT_[:, g, (t0 + i) * 128:(t0 + i + 1) * 128],
                                        QT[:, g, j * FB:(j + 1) * FB],
                                        start=True, stop=True,
                                    )
                                ex = p2e.tile([128, w, FB], BF16, tag="exp", name="exp")
                                nc.scalar.activation(
                                    ex[:], ps_sq[:],
                                    mybir.ActivationFunctionType.Exp,
                                    scale=0.125,
                                )
                                nc.vector.tensor_mul(
                                    ex[:], ex[:], mask_j[:, t0:t0 + w, :]
                                )
                                if pend is not None:
                                    pt0, pw, pex = pend
                                    for i in range(pw):
                                        ti = pt0 + i
                                        nc.tensor.matmul(
                                            ps_ctx[:],
                                            Vsb[:, ti, n * HP1:(n + 1) * HP1],
                                            pex[:, i, :],
                                            start=(ti == 0), stop=False,
                                        )
                                pend = (t0, w, ex)
                                t0 += w
                            pt0, pw, pex = pend
                            for i in range(pw):
                                ti = pt0 + i
                                nc.tensor.matmul(
                                    ps_ctx[:],
                                    Vsb[:, ti, n * HP1:(n + 1) * HP1],
                                    pex[:, i, :],
                                    start=False, stop=(ti == NT - 1),
                                )
                            ctx_sb = p2s.tile([HP1, FB], F32, tag="ctx_sb",
                                              name="ctx_sb", bufs=5)
                            nc.vector.tensor_copy(ctx_sb[:], ps_ctx[:])
                            # gather this head's sums row onto partition n
                            nc.gpsimd.dma_start(
                                sums_g[n:n + 1, :], ctx_sb[H:H + 1, :]
                            )
                            ctx_keep.append((n, ctx_sb))
                            if n in (0, 1) and pending_norm is not None:
                                pending_norm(n)
                                if n == 1:
                                    pending_norm = None
                        # batched normalization for this j's 4 heads --
                        # deferred into the next j's stream so the long
                        # reciprocal doesn't block the DVE queue at the
                        # j boundary
                        def _norm(step, j=j, sums_g=sums_g, recip=recip,
                                  recip_h=recip_h, ctx_keep=list(ctx_keep)):
                            CH = FB // 4
                            if step == 0:
                                for c in range(2):
                                    nc.vector.reciprocal(
                                        recip[0:HPC, c * CH:(c + 1) * CH],
                                        sums_g[0:HPC, c * CH:(c + 1) * CH],
                                    )
                                return
                            for c in range(2, 4):
                                nc.vector.reciprocal(
                                    recip[0:HPC, c * CH:(c + 1) * CH],
                                    sums_g[0:HPC, c * CH:(c + 1) * CH],
                                )
                            nc.vector.tensor_copy(recip_h[0:HPC, :], recip[0:HPC, :])
                            for nn, ctx_sb in ctx_keep:
                                ps_bc = ps_b.tile([128, FB], F32, tag="bc",
                                                  name="ps_bc")
                                nc.tensor.matmul(
                                    ps_bc[:], ones_bc_sb[:, nn, :], recip_h[:],
                                    start=True, stop=True,
                                )
                                out_sb = p2s.tile([H, FB], F32, tag="out")
                                nc.vector.tensor_mul(
                                    out_sb[:], ctx_sb[0:H, :], ps_bc[0:H, :]
                                )
                                nc.gpsimd.dma_start(
                                    out_ctx[nn, :, j * FB:(j + 1) * FB],
                                    out_sb[:],
                                )
                        pending_norm = _norm
                    pending_norm(0)
                    pending_norm(1)
                p1_cm.__exit__(None, None, None)
                ps1_cm.__exit__(None, None, None)

    nc.compile()
    return nc


_compiled = None


def _get_compiled():
    global _compiled
    if _compiled is None:
        _compiled = _program()
    return _compiled


def make_in_maps(from_tensor, to_tensor, attention_mask, wq, bq, wk, bk, wv, bv):
    bf = ml_dtypes.bfloat16
    from_tensor = np.asarray(from_tensor, dtype=np.float32)
    to_tensor = np.asarray(to_tensor, dtype=np.float32)
    attention_mask = np.asarray(attention_mask)
    wq = np.asarray(wq, dtype=np.float32)
    wk = np.asarray(wk, dtype=np.float32)
    wv = np.asarray(wv, dtype=np.float32)
    bq = np.asarray(bq, dtype=np.float32)
    bk = np.asarray(bk, dtype=np.float32)
    bv = np.asarray(bv, dtype=np.float32)

    fromT_b = [np.ascontiguousarray(from_tensor[b].T).astype(bf) for b in range(B)]
    toT_b = [np.ascontiguousarray(to_tensor[b].T).astype(bf) for b in range(B)]
    maskT_b = [attention_mask[b].T.astype(bf) for b in range(B)]
    vones_arr = np.zeros((128, 128), dtype=bf)
    vones_arr[0, :] = 1.0
    ones_bc_arr = np.zeros((128, HPC, 128), dtype=np.float16)
    for nn in range(HPC):
        ones_bc_arr[nn, nn, :] = 1.0

    in_maps = []
    for c in range(NCORES):
        b, hb = divmod(c, NCORES // B)
        hs = hb * HPC
        bq_dev = bq[hs:hs + HPC].reshape(NG, 128).T
        bk_dev = bk[hs:hs + HPC].reshape(NG, 128).T
        bv_pad = np.zeros((128, HPC * H), dtype=bf)
        bv_pad[0, :] = bv[hs:hs + HPC].reshape(HPC * H)
        in_maps.append(
            dict(
                fromT=fromT_b[b],
                toT=toT_b[b],
                maskT=maskT_b[b],
                wq=wq[:, hs:hs + HPC, :].reshape(D, HPC * H).astype(bf),
                wk=wk[:, hs:hs + HPC, :].reshape(D, HPC * H).astype(bf),
                wv=wv[:, hs:hs + HPC, :].reshape(D, HPC * H).astype(bf),
                bqk=np.ascontiguousarray(
                    np.concatenate([bq_dev, bk_dev], axis=1), dtype=np.float32
                ),
                bv_pad=bv_pad,
                vones=vones_arr,
                ones_bc=ones_bc_arr,
            )
        )
    return in_maps


def gather_output(results):
    out = np.empty((B, F, N, H), dtype=np.float32)
    for c in range(NCORES):
        b, hb = divmod(c, NCORES // B)
        hs = hb * HPC
        ctx = results[c]["out_ctx"]  # [HPC, H, F]
        out[b, :, hs:hs + HPC, :] = ctx.transpose(2, 0, 1)
    return out


def run_sharded(inputs, **run_kwargs):
    """Run the SPMD kernel; returns (output, BassKernelResults)."""
    nc = _get_compiled()
    in_maps = make_in_maps(**inputs)
    res = run_bass_kernel_spmd(nc, in_maps, list(range(NCORES)), **run_kwargs)
    return gather_output(res.results), res


def kernel(**inputs):
    out, _ = run_sharded(inputs)
    return out
